# revision 2
# baseline (speedup 1.0000x reference)
"""BEV-pool (segment-sum scatter) Trainium2 kernel for nn_BaseDepthTransform.

Strategy:
  Host (numpy): replicate the reference geometry -> per-point flat BEV bin id
  (depends only on the small camera matrices, not on x). Sort points by bin,
  cut into 128-point tiles confined to 128-bin aligned windows, group tiles
  into fixed-size PSUM accumulation groups (K=8 bulk + K=1 remainder), and
  split windows across the 8 NeuronCores balanced by tile count.

  Device (Bass/Tile, SPMD over 8 cores): stream the scheduled feature tiles,
  build a one-hot matrix per tile with a DVE is_equal against an iota ramp,
  matmul (one-hot^T @ feats) to segment-sum each tile into its 128-bin PSUM
  window, chain 8 tiles per PSUM accumulation where possible, then add the
  PSUM window into a per-core SBUF slab at a register-dynamic offset.
  Finally DMA the slab out; host scatters per-core slabs into the full grid.

  No collectives: each 128-bin window is owned by exactly one core.
"""
import sys
sys.path.insert(0, '/opt/trn_rl_repo')

import numpy as np

# ---- static problem config (mirrors the reference) ----
IH, IW = 256, 704
FH, FW = 32, 88
D = 118
C = 80
NXg, NYg, NZg = 360, 360, 1
BXc = np.array([-53.85, -53.85, 0.0], np.float32)
DXc = np.array([0.3, 0.3, 20.0], np.float32)
NBINS = NZg * NXg * NYg  # 129600
WIN = 128                # bins per window (aligned)
KA = 8                   # tiles per phase-A PSUM accumulation group
NCORES = 8
DMA_GRP = 8              # tiles per feature DMA

_BUILD_CACHE = {}


def _frustum():
    ds = np.arange(1.0, 60.0, 0.5, dtype=np.float32)
    xs = np.linspace(0.0, IW - 1.0, FW, dtype=np.float32)
    ys = np.linspace(0.0, IH - 1.0, FH, dtype=np.float32)
    ds_g = np.broadcast_to(ds[:, None, None], (D, FH, FW))
    xs_g = np.broadcast_to(xs[None, None, :], (D, FH, FW))
    ys_g = np.broadcast_to(ys[None, :, None], (D, FH, FW))
    return np.stack([xs_g, ys_g, ds_g], axis=-1)  # [D,FH,FW,3]


def _get_geometry(c2l_rots, c2l_trans, intrins, post_rots, post_trans,
                  extra_rots, extra_trans):
    fr = _frustum()
    pts = fr[None, None] - post_trans[:, :, None, None, None, :]
    inv_pr = np.linalg.inv(post_rots).astype(np.float32)
    pts = np.einsum('bnij,bndhwj->bndhwi', inv_pr, pts).astype(np.float32)
    pts = np.concatenate([pts[..., :2] * pts[..., 2:3], pts[..., 2:3]], axis=-1)
    combine = np.einsum(
        'bnij,bnjk->bnik', c2l_rots, np.linalg.inv(intrins).astype(np.float32)
    ).astype(np.float32)
    pts = np.einsum('bnij,bndhwj->bndhwi', combine, pts).astype(np.float32)
    pts = pts + c2l_trans[:, :, None, None, None, :]
    pts = np.einsum('bij,bndhwj->bndhwi', extra_rots, pts).astype(np.float32)
    pts = pts + extra_trans[:, None, None, None, None, :]
    return pts  # [B,N,D,FH,FW,3]


def _flat_bins(geom):
    """Per-point flat bin id (int64), -1 for dropped points."""
    coords = ((geom - (BXc - DXc / 2.0)) / DXc).astype(np.int32)
    B = coords.shape[0]
    coords = coords.reshape(B, -1, 3)
    cx, cy, cz = coords[..., 0], coords[..., 1], coords[..., 2]
    kept = (cx >= 0) & (cx < NXg) & (cy >= 0) & (cy < NYg) & (cz >= 0) & (cz < NZg)
    flat = ((cz.astype(np.int64) * NXg + cx) * NYg + cy)
    flat = np.where(kept, flat, -1)
    return flat  # [B, Np]


def _round_up(x, m):
    return ((x + m - 1) // m) * m


def _build_schedule(fk_sorted):
    """Split windows across cores; return per-core window/tile structure and
    the rounded uniform shape params (NA, NB, WMAX)."""
    wid = fk_sorted // WIN
    uw, w_start, w_cnt = np.unique(wid, return_index=True, return_counts=True)
    tiles_w = (w_cnt + 127) // 128
    a_w = (tiles_w // KA) * KA          # phase-A tiles per window
    r_w = tiles_w - a_w                 # phase-B tiles per window

    cum_tiles = np.cumsum(tiles_w)
    total_tiles = int(cum_tiles[-1]) if len(cum_tiles) else 0
    bounds = [0]
    for k in range(1, NCORES):
        t = k * total_tiles / NCORES
        bounds.append(int(np.searchsorted(cum_tiles, t, side='left') + 1))
    bounds.append(len(uw))
    for i in range(1, len(bounds)):
        bounds[i] = min(max(bounds[i], bounds[i - 1]), len(uw))

    cores = []
    for ci in range(NCORES):
        lo, hi = bounds[ci], bounds[ci + 1]
        cores.append({
            'win_ids': uw[lo:hi],
            'w_start': w_start[lo:hi],
            'w_cnt': w_cnt[lo:hi],
            'tiles_w': tiles_w[lo:hi],
            'a_w': a_w[lo:hi],
        })

    NA = max(1, max((int(c['a_w'].sum()) // KA for c in cores), default=1))
    NB = max(1, max((int((c['tiles_w'] - c['a_w']).sum()) for c in cores),
                    default=1))
    NA = _round_up(NA, 4)
    NB = _round_up(NB, DMA_GRP)
    WMAX = _round_up(max(1, max((len(c['win_ids']) for c in cores), default=1)), 4)
    return cores, NA, NB, WMAX


def _build_core_inputs(core, fk_sorted, pidx_sorted, xflat, NA, NB):
    """Build feats stream, ids and qv arrays for one core (padded shapes)."""
    TA = NA * KA
    T = TA + NB

    pidx = np.full((T, 128), -1, np.int64)
    lids = np.full((T, 128), -1.0, np.float32)
    qv = np.zeros((NA + NB,), np.int32)

    ai = 0  # phase-A tile cursor
    bi = 0  # phase-B tile cursor
    nwin = len(core['win_ids'])
    for iw in range(nwin):
        o = int(core['w_start'][iw])
        cnt = int(core['w_cnt'][iw])
        tw = int(core['tiles_w'][iw])
        aw = int(core['a_w'][iw])
        for j in range(tw):
            s = o + j * 128
            ln = min(128, cnt - j * 128)
            if j < aw:
                t = ai
                if j % KA == 0:
                    qv[ai // KA] = iw * C
                ai += 1
            else:
                t = TA + bi
                qv[NA + bi] = iw * C
                bi += 1
            pidx[t, :ln] = pidx_sorted[s:s + ln]
            lids[t, :ln] = (fk_sorted[s:s + ln] % WIN).astype(np.float32)
    assert ai <= TA and bi <= NB

    ngrp = T // DMA_GRP
    feats = np.zeros((T, 128, C), np.float32)
    v = pidx >= 0
    feats[v] = xflat[pidx[v]]
    feats = feats.reshape(ngrp, DMA_GRP, 128, C).transpose(0, 2, 1, 3) \
                 .reshape(ngrp, 128, DMA_GRP * C)
    return {
        'feats': np.ascontiguousarray(feats),
        'ids': np.ascontiguousarray(lids.T),
        'qv': np.ascontiguousarray(qv[None, :]),
    }


def _build_bass(NA, NB, WMAX):
    """Build + finalize the SPMD Bass graph for given schedule shape."""
    key = (NA, NB, WMAX)
    if key in _BUILD_CACHE:
        return _BUILD_CACHE[key]
    from concourse import bass, mybir, tile, bacc

    TA = NA * KA
    T = TA + NB
    ngrp = T // DMA_GRP

    nc = bacc.Bacc()
    feats = nc.declare_dram_parameter("feats", [ngrp, 128, DMA_GRP * C],
                                      mybir.dt.float32, isOutput=False)
    ids = nc.declare_dram_parameter("ids", [128, T], mybir.dt.float32,
                                    isOutput=False)
    qv = nc.declare_dram_parameter("qv", [1, NA + NB], mybir.dt.int32,
                                   isOutput=False)
    out = nc.declare_dram_parameter("out", [128, WMAX * C], mybir.dt.float32,
                                    isOutput=True)

    with tile.TileContext(nc) as tc:
        with tc.tile_pool(name="persist", bufs=1) as ppool, \
             tc.tile_pool(name="stream", bufs=4) as pool, \
             tc.tile_pool(name="oh", bufs=4) as ohpool, \
             tc.tile_pool(name="psum", bufs=4, space="PSUM") as psum_pool:
            iota_i = ppool.tile([128, 128], mybir.dt.int32)
            nc.gpsimd.iota(iota_i[:], pattern=[[1, 128]], channel_multiplier=0)
            iota_f = ppool.tile([128, 128], mybir.dt.float32)
            nc.vector.tensor_copy(iota_f[:], iota_i[:])

            ids_sb = ppool.tile([128, T], mybir.dt.float32)
            nc.sync.dma_start(ids_sb[:], ids[:, :])
            qv_sb = ppool.tile([1, NA + NB], mybir.dt.int32)
            nc.sync.dma_start(qv_sb[:], qv[:, :])

            slab = ppool.tile([128, WMAX * C], mybir.dt.float32)
            nc.vector.memset(slab[:], 0.0)

            def flush(gi, ptile):
                q = nc.values_load(
                    qv_sb[0:1, gi:gi + 1],
                    engines=[mybir.EngineType.DVE],
                    min_val=0, max_val=(WMAX - 1) * C,
                    skip_runtime_bounds_check=True,
                )
                nc.vector.tensor_tensor(
                    out=slab[:, bass.ds(q, C)],
                    in0=slab[:, bass.ds(q, C)],
                    in1=ptile[:],
                    op=mybir.AluOpType.add,
                )

            def onehot_of(t):
                oh = ohpool.tile([128, 128], mybir.dt.float32, tag="oh")
                nc.vector.tensor_tensor(
                    out=oh[:],
                    in0=ids_sb[:, t:t + 1].to_broadcast((128, 128)),
                    in1=iota_f[:],
                    op=mybir.AluOpType.is_equal,
                )
                return oh

            # ---- phase A: KA-tile PSUM accumulation groups ----
            for g in range(NA):
                chunk = pool.tile([128, DMA_GRP * C], mybir.dt.float32,
                                  tag="chunk")
                nc.sync.dma_start(chunk[:], feats[g, :, :])
                ptile = psum_pool.tile([128, C], mybir.dt.float32)
                for k in range(KA):
                    t = g * KA + k
                    oh = onehot_of(t)
                    nc.tensor.matmul(
                        out=ptile[:], lhsT=oh[:],
                        rhs=chunk[:, k * C:(k + 1) * C],
                        start=(k == 0), stop=(k == KA - 1),
                    )
                flush(g, ptile)

            # ---- phase B: single-tile groups ----
            for bg in range(NB // DMA_GRP):
                chunk = pool.tile([128, DMA_GRP * C], mybir.dt.float32,
                                  tag="chunk")
                nc.sync.dma_start(chunk[:], feats[NA + bg, :, :])
                for k in range(DMA_GRP):
                    j = bg * DMA_GRP + k
                    oh = onehot_of(TA + j)
                    ptile = psum_pool.tile([128, C], mybir.dt.float32)
                    nc.tensor.matmul(
                        out=ptile[:], lhsT=oh[:],
                        rhs=chunk[:, k * C:(k + 1) * C],
                        start=True, stop=True,
                    )
                    flush(NA + j, ptile)

            nc.sync.dma_start(out[:, :], slab[:, :])
    nc.finalize()
    _BUILD_CACHE[key] = nc
    return nc


def run_scheduled(x, flat, trace=False, trace_cores=None):
    """Core pipeline given precomputed flat bins; returns (grid, results)."""
    from concourse.bass_utils import run_bass_kernel_spmd

    xflat = np.ascontiguousarray(x.reshape(-1, C))
    kept_idx = np.nonzero(flat >= 0)[0]
    fk = flat[kept_idx]
    order = np.argsort(fk, kind='stable')
    fk_sorted = fk[order]
    pidx_sorted = kept_idx[order]

    cores, NA, NB, WMAX = _build_schedule(fk_sorted)
    in_maps = [
        _build_core_inputs(cores[ci], fk_sorted, pidx_sorted, xflat, NA, NB)
        for ci in range(NCORES)
    ]
    nc = _build_bass(NA, NB, WMAX)
    res = run_bass_kernel_spmd(nc, in_maps, core_ids=list(range(NCORES)),
                               trace=trace, trace_cores=trace_cores)

    nwin_total = (NBINS + WIN - 1) // WIN
    grid = np.zeros((nwin_total * WIN, C), np.float32)
    for ci in range(NCORES):
        slab = res.results[ci]["out"]   # [128, WMAX*C]
        for iw, w in enumerate(cores[ci]['win_ids']):
            grid[w * WIN:(w + 1) * WIN] += slab[:, iw * C:(iw + 1) * C]
    return grid[:NBINS], res


def kernel(x, camera2lidar_rots, camera2lidar_trans, intrins, post_rots,
           post_trans, extra_rots, extra_trans):
    x = np.asarray(x, np.float32)
    B, N = x.shape[0], x.shape[1]
    assert (B, N) == (1, 6) and x.shape[2:] == (D, FH, FW, C), x.shape

    geom = _get_geometry(
        np.asarray(camera2lidar_rots, np.float32),
        np.asarray(camera2lidar_trans, np.float32),
        np.asarray(intrins, np.float32),
        np.asarray(post_rots, np.float32),
        np.asarray(post_trans, np.float32),
        np.asarray(extra_rots, np.float32),
        np.asarray(extra_trans, np.float32),
    )
    flat = _flat_bins(geom)[0]          # [Np]
    grid, _ = run_scheduled(x, flat)
    outp = grid.reshape(NXg, NYg, C).transpose(2, 0, 1)[None]  # [1,C,NX,NY]
    return np.ascontiguousarray(outp)
